# revision 36
# baseline (speedup 1.0000x reference)
"""Bilateral filter (d=7, sigma_color=0.1, sigma_space=3.0) on 8 Trainium2 cores.

Input x: [16, 3, 768, 768] fp32.  out = sum_{(i,j)!=0, |i|,|j|<=7} sw[i,j] *
exp(-50*(s_ij - x)^2) * s_ij  with s_ij the reflect-padded shifted window.

Strategy (per core = 2 images x 3 channels = 6 planes, data-parallel):
- Partitions carry (plane, row-strip): 6 planes x 21 strips of 37 rows = 126
  partitions. Both spatial dims live in the free dimension so the (i,j)
  window shifts are plain strided AP reads.
- Host reflect-pads each plane to [791, 782] fp16 (halves DMA bytes and
  makes every DVE op a 16-bit 2x-mode op).
- Symmetric pairs: for (i,j) and (-i,-j), diff fields are negatives and the
  color weight is even, so ONE subtract + ONE Derivative_Erf on an extended
  (SH+i) x (WC+|j|) domain serves both offsets.
- exp(-50 d^2) via ACT Derivative_Erf (= 2/sqrt(pi) exp(-y^2), y=sqrt(50) d).
- Per-offset multiply t = e * win on DVE fp16 (2x mode, alignment kept 4B via
  even/odd-shifted fp16 copies of the slab).  A tunable fraction of the
  subtracts runs on GPSIMD (Pool) to unload DVE.
- Accumulation over the 224 (or fewer, see r2_cut) offsets on TensorE:
  psum += sid[g].T @ t with sid[g] = (sw_ij*sqrt(pi)/2) * I.  PSUM holds the
  full [37*96] chunk as row-chunks; output DMAs straight from PSUM.
- r2_cut: offsets with i^2+j^2 >= r2_cut are dropped (they carry <1% of the
  spatial-kernel mass for r2_cut=65; measured end-to-end rel-err stays well
  inside the 2e-2 gate).
"""
import numpy as np

D = 7
SIGMA_COLOR = 0.1
SIGMA_SPACE = 3.0

N_CORES = 8
PLANES = 6            # per-core planes (2 images x 3 channels)
STRIPS = 21           # row-strips per plane
SH = 37               # strip height -> 21*37 = 777 >= 768
P_USED = PLANES * STRIPS   # 126 partitions
H = W = 768
HP = STRIPS * SH      # 777 padded output rows per plane
XROWS = SH * (STRIPS - 1) + SH + 2 * D  # 791 input rows needed per plane
XCOLS = W + 2 * D     # 782
WC = 96               # column chunk width
NCHUNK = W // WC      # 8
CHUNK_F = SH * WC     # 3552 output elems per partition per chunk
SLABC = 112           # sbuf slab cols (110 used + 2 zero pad)

_CACHE = {}


def _sw_table():
    offs = np.arange(-D, D + 1)
    sw = np.exp(-0.5 * (offs[:, None] ** 2 + offs[None, :] ** 2) / SIGMA_SPACE ** 2)
    return (sw / sw.sum()).astype(np.float32)


def build(reps=1, gp_frac=0.0, r2_cut=72, mm_big=False, deep=False,
          batch2=False, tt3=True, prod5=False):
    """gp_frac: fraction of pair-subtract work offloaded to GPSIMD.
    r2_cut: keep pair (i,j) iff i*i+j*j < r2_cut (226 = keep all).
    mm_big: True -> matmul N up to 960 spanning 2 PSUM banks; False -> N<=480.
    batch2: batch same-i same-sign same-parity j-pairs (2 per subtract/derf
    instruction via a stride-0/-2 batch dim) to amortize per-op overhead.
    """
    import concourse.tile as tile
    import concourse.bass as bass
    from concourse import bacc, mybir

    f32 = mybir.dt.float32
    fp16 = mybir.dt.float16

    nc = bacc.Bacc("TRN2", target_bir_lowering=False, debug=False,
                   num_devices=N_CORES)
    xp = nc.dram_tensor("xp", [PLANES, XROWS, XCOLS], fp16, kind="ExternalInput")
    out = nc.dram_tensor("out", [P_USED * SH, W], fp16, kind="ExternalOutput")

    sw = _sw_table()
    CDERF = float(np.sqrt(np.pi) / 2.0)
    SCALE = float(np.sqrt(0.5 / SIGMA_COLOR ** 2))  # sqrt(50)

    # symmetric pair list (one of each +/- pair), radius-cut, sorted by group
    pairs = [(i, j) for i in range(0, D + 1) for j in range(-D, D + 1)
             if ((i > 0) or (i == 0 and j > 0)) and (i * i + j * j) < r2_cut]
    cvals = sorted({float(sw[D + i, D + j]) for (i, j) in pairs})
    NSW = len(cvals)
    gidx_of = {p: cvals.index(float(sw[D + p[0], D + p[1]])) for p in pairs}
    NP = len(pairs)

    # group pairs for batched subtract/derf: same i, same sign(j), same
    # parity(j); within a group the in0/in1 col bases step by 0 / -2.
    if batch2:
        from collections import defaultdict
        buckets = defaultdict(list)
        singles = []
        for (i, j) in pairs:
            if j == 0:
                singles.append([(i, j)])
            else:
                buckets[(i, j > 0, abs(j) % 2)].append((i, j))
        groups = []
        for v in buckets.values():
            v.sort(key=lambda p: abs(p[1]))
            while len(v) >= 2:
                groups.append(v[:2])
                v = v[2:]
            if v:
                groups.append(v)
        groups += singles
        groups.sort(key=lambda g: g[0][0])
    else:
        groups = [[p] for p in sorted(pairs, key=lambda p: gidx_of[p])]

    # psum row-chunking
    if mm_big:
        RCH, BANKW, NBANK = 10, 1024, 4   # rows/chunk, psum slot width, slots
    else:
        RCH, BANKW, NBANK = 5, 512, 8
    chunks = []
    r0 = 0
    while r0 < SH:
        chunks.append((r0, min(RCH, SH - r0)))
        r0 += RCH
    NMM_PER_ACC = len(chunks)

    out3 = out.ap().rearrange("(p r) w -> p r w", r=SH)

    with tile.TileContext(nc) as tc:
        with (
            tc.tile_pool(name="consts", bufs=1) as consts,
            tc.tile_pool(name="s16e_p", bufs=2) as s16e_p,
            tc.tile_pool(name="s16o_p", bufs=1 if batch2 else 2) as s16o_p,
            tc.tile_pool(name="dext_p",
                         bufs=2 if batch2 else (4 if deep else 3)) as dext_p,
            tc.tile_pool(name="e_p",
                         bufs=2 if batch2 else (4 if deep else 3)) as e_p,
            tc.tile_pool(name="prod_p",
                         bufs=2 if batch2 else (5 if prod5 else 4)) as prod_p,
            tc.tile_pool(name="tt_p", bufs=3 if tt3 else 2) as tt_p,
            tc.tile_pool(name="outb_p",
                         bufs=1 if (batch2 or prod5) else 2) as outb_p,
            tc.tile_pool(name="psum_p", bufs=1, space="PSUM") as psum_p,
        ):
            sid = consts.tile([128, NSW, 128], fp16)
            nc.gpsimd.memset(sid[:], 0.0)
            for g in range(NSW):
                nc.gpsimd.affine_select(
                    out=sid[:, g, :], in_=sid[:, g, :],
                    compare_op=mybir.AluOpType.not_equal,
                    fill=cvals[g] * CDERF, base=0,
                    pattern=[[-1, 128]], channel_multiplier=1)

            def s16ap(se, so, row0, nrows, col0, ncols):
                """Slab fp16 window [row0:row0+nrows, col0:col0+ncols] with a
                4B-aligned start: even col0 reads se, odd reads so (shift 1)."""
                if col0 % 2 == 0:
                    return se[0:P_USED, row0:row0 + nrows, col0:col0 + ncols]
                return so[0:P_USED, row0:row0 + nrows, col0 - 1:col0 - 1 + ncols]

            def body(_iv=None):
                for t in range(NCHUNK):
                    se = s16e_p.tile([128, SH + 2 * D, SLABC], fp16, tag="se")
                    for c in range(PLANES):
                        src = bass.AP(
                            tensor=xp, offset=c * XROWS * XCOLS + WC * t,
                            ap=[[SH * XCOLS, STRIPS], [XCOLS, SH + 2 * D],
                                [1, WC + 2 * D]])
                        nc.sync.dma_start(
                            out=se[STRIPS * c:STRIPS * (c + 1), :, 0:WC + 2 * D],
                            in_=src)
                    nc.gpsimd.memset(se[0:P_USED, :, WC + 2 * D:SLABC], 0.0)
                    so = s16o_p.tile([128, SH + 2 * D, SLABC], fp16, tag="so")
                    nc.scalar.copy(so[0:P_USED, :, 0:SLABC - 2],
                                   se[0:P_USED, :, 1:SLABC - 1])

                    psum = psum_p.tile([128, NBANK, BANKW], f32, tag="psum")

                    gp_acc = 0.0
                    nmm = 0
                    last_mm = NP * 2 * NMM_PER_ACC
                    for grp in groups:
                        gi = grp[0][0]
                        gn = len(grp)
                        ER = SH + gi
                        ec2s = [(WC + abs(j)) + ((WC + abs(j)) & 1)
                                for (_, j) in grp]
                        EC2m = max(ec2s)

                        # ---- subtract: dext[s] = x(q+o_s) - x(q), batched
                        nslot = 2 if batch2 else 1
                        dext = dext_p.tile([128, nslot, SH + D, WC + D + 1],
                                           fp16, tag="dext")
                        gp_acc += gp_frac * gn
                        if gp_acc >= 1.0:
                            gp_acc -= 1.0
                            sub_eng = nc.gpsimd
                        else:
                            sub_eng = nc.vector
                        c0s = [D + min(j, 0) for (_, j) in grp]
                        c1s = [D - max(j, 0) for (_, j) in grp]
                        b0 = s16ap(se, so, D, ER, c0s[0], EC2m)
                        b1 = s16ap(se, so, D - gi, ER, c1s[0], EC2m)
                        if gn == 1:
                            in0, in1 = b0, b1
                        else:
                            d0 = c0s[1] - c0s[0]
                            d1 = c1s[1] - c1s[0]
                            in0 = bass.AP(tensor=b0.tensor, offset=b0.offset,
                                          ap=[b0.ap[0], [d0, gn],
                                              b0.ap[1], b0.ap[2]])
                            in1 = bass.AP(tensor=b1.tensor, offset=b1.offset,
                                          ap=[b1.ap[0], [d1, gn],
                                              b1.ap[1], b1.ap[2]])
                        sub_eng.tensor_tensor(
                            dext[0:P_USED, 0:gn, 0:ER, 0:EC2m], in0, in1,
                            mybir.AluOpType.subtract)

                        # ---- color weight: e = 2/sqrt(pi) exp(-50 dext^2)
                        eb = e_p.tile([128, nslot, SH + D, WC + D + 1], fp16,
                                      tag="e")
                        nc.scalar.activation(
                            eb[0:P_USED, 0:gn, 0:ER, 0:EC2m],
                            dext[0:P_USED, 0:gn, 0:ER, 0:EC2m],
                            mybir.ActivationFunctionType.Derivative_Erf,
                            scale=SCALE)

                        for sl, (i, j) in enumerate(grp):
                            ER = SH + i
                            EC = WC + abs(j)
                            EC2 = EC + (EC & 1)
                            jp, jn = max(j, 0), max(-j, 0)
                            g = gidx_of[(i, j)]
                            lhsT = sid[0:P_USED, g, 0:P_USED]
                            e = eb[0:P_USED, sl]
                            if j % 2 == 0:
                                # dense path: both products into [2,SH,WC]
                                # (e read at even col offsets jp/jn -> 2x)
                                tt = tt_p.tile([128, 2, SH, WC], fp16,
                                               tag="tt")
                                nc.vector.tensor_tensor(
                                    tt[0:P_USED, 0],
                                    e[0:P_USED, i:i + SH, jp:jp + WC],
                                    s16ap(se, so, D + i, SH, D + j, WC),
                                    mybir.AluOpType.mult)
                                nc.vector.tensor_tensor(
                                    tt[0:P_USED, 1],
                                    e[0:P_USED, 0:SH, jn:jn + WC],
                                    s16ap(se, so, D - i, SH, D - j, WC),
                                    mybir.AluOpType.mult)
                                ttf = tt.rearrange("p s r c -> p s (r c)")
                                for s2 in range(2):
                                    for m, (mr0, mnr) in enumerate(chunks):
                                        n = mnr * WC
                                        nc.tensor.matmul(
                                            psum[0:P_USED, m, 0:n], lhsT,
                                            ttf[0:P_USED, s2,
                                                mr0 * WC:mr0 * WC + n],
                                            start=(nmm < NMM_PER_ACC),
                                            stop=(nmm >= last_mm - NMM_PER_ACC))
                                        nmm += 1
                            else:
                                # ext-product path: products on the extended
                                # domain keep every DVE operand 4B-aligned;
                                # matmul rhs reads strided rows.
                                # t+(r,c)=P(r+i,c+jp), P(a,b)=e(a,b)*x(a,b-jn)
                                # t-(r,c)=Q(r,c+jn), Q(a,b)=e(a,b)*x(a-i,b-jp-jn)
                                pp = prod_p.tile([128, SH + D, WC + D + 1],
                                                 fp16, tag="pq")
                                nc.vector.tensor_tensor(
                                    pp[0:P_USED, 0:ER, 0:EC2],
                                    e[0:P_USED, 0:ER, 0:EC2],
                                    s16ap(se, so, D, ER, D - jn, EC2),
                                    mybir.AluOpType.mult)
                                qq = prod_p.tile([128, SH + D, WC + D + 1],
                                                 fp16, tag="pq")
                                nc.vector.tensor_tensor(
                                    qq[0:P_USED, 0:ER, 0:EC2],
                                    e[0:P_USED, 0:ER, 0:EC2],
                                    s16ap(se, so, D - i, ER, D - j - jn, EC2),
                                    mybir.AluOpType.mult)
                                for src, sr0, sc0 in ((pp, i, jp), (qq, 0, jn)):
                                    for m, (mr0, mnr) in enumerate(chunks):
                                        n = mnr * WC
                                        nc.tensor.matmul(
                                            psum[0:P_USED, m, 0:n], lhsT,
                                            src[0:P_USED,
                                                sr0 + mr0:sr0 + mr0 + mnr,
                                                sc0:sc0 + WC],
                                            start=(nmm < NMM_PER_ACC),
                                            stop=(nmm >= last_mm - NMM_PER_ACC))
                                        nmm += 1

                    # ---- output: evacuate PSUM -> SBUF (dense rows), DMA out
                    nfull = (SH // RCH)  # full row-chunks
                    r0, nr = chunks[-1]
                    outb = outb_p.tile([128, NBANK, RCH * WC], fp16, tag="outb")
                    nc.scalar.copy(outb[0:P_USED, 0:nfull, :],
                                   psum[0:P_USED, 0:nfull, 0:RCH * WC])
                    nc.scalar.copy(outb[0:P_USED, nfull, 0:nr * WC],
                                   psum[0:P_USED, nfull, 0:nr * WC])
                    nc.sync.dma_start(
                        out=out3[:, 0:nfull * RCH, WC * t:WC * t + WC],
                        in_=outb[0:P_USED, 0:nfull, :])
                    nc.sync.dma_start(
                        out=out3[:, r0:r0 + nr, WC * t:WC * t + WC],
                        in_=outb[0:P_USED, nfull, 0:nr * WC].rearrange(
                            "p (r c) -> p r c", c=WC))

            if reps == 1:
                body()
            else:
                with tc.For_i(0, reps, 1) as _i:
                    body(_i)
    nc.compile()
    return nc


def _prepare_inputs(x):
    """x: [16,3,768,768] fp32 -> per-core fp16 padded plane stacks [6,791,782]."""
    planes = np.ascontiguousarray(x.reshape(N_CORES, PLANES, H, W)).astype(np.float16)
    in_maps = []
    for c in range(N_CORES):
        xp = np.pad(planes[c], ((0, 0), (D, D + (XROWS - H - 2 * D)), (D, D)),
                    mode="reflect")
        in_maps.append({"xp": xp})
    return in_maps


def _gather_outputs(results):
    outs = []
    for c in range(N_CORES):
        o = results[c]["out"].reshape(PLANES, HP, W)[:, :H, :]
        outs.append(o)
    return np.stack(outs).reshape(16, 3, H, W).astype(np.float32)


def _build_runner(nc):
    """Cached jit(shard_map(bass_exec)) runner over the 8 axon cores, same
    mechanics as bass2jax.run_bass_via_pjrt but reusable across calls (no
    per-call retrace) and with donated zero outputs created on-device."""
    import jax
    import jax.numpy as jnp
    from jax.sharding import Mesh, PartitionSpec, NamedSharding
    from jax.experimental.shard_map import shard_map
    from concourse import mybir
    from concourse.bass2jax import _bass_exec_p, install_neuronx_cc_hook
    install_neuronx_cc_hook()

    partition_name = nc.partition_id_tensor.name if nc.partition_id_tensor else None
    in_names, out_names, out_avals, out_np = [], [], [], []
    for alloc in nc.m.functions[0].allocations:
        if not isinstance(alloc, mybir.MemoryLocationSet):
            continue
        name = alloc.memorylocations[0].name
        if alloc.kind == "ExternalInput":
            if name != partition_name:
                in_names.append(name)
        elif alloc.kind == "ExternalOutput":
            shape = tuple(alloc.tensor_shape)
            dtype = mybir.dt.np(alloc.dtype)
            out_names.append(name)
            out_avals.append(jax.core.ShapedArray(shape, dtype))
            out_np.append((shape, dtype))
    n_params = len(in_names)
    all_in_names = list(in_names) + list(out_names)
    if partition_name is not None:
        all_in_names.append(partition_name)

    def _body(*args):
        operands = list(args)
        if partition_name is not None:
            from concourse.bass2jax import partition_id_tensor
            operands.append(partition_id_tensor())
        outs = _bass_exec_p.bind(
            *operands,
            out_avals=tuple(out_avals),
            in_names=tuple(all_in_names),
            out_names=tuple(out_names),
            lowering_input_output_aliases=(),
            sim_require_finite=True,
            sim_require_nnan=True,
            nc=nc,
        )
        return tuple(outs)

    devices = jax.devices()[:N_CORES]
    mesh = Mesh(np.asarray(devices), ("core",))
    sh = NamedSharding(mesh, PartitionSpec("core"))
    n_outs = len(out_avals)
    in_specs = (PartitionSpec("core"),) * (n_params + n_outs)
    out_specs = (PartitionSpec("core"),) * n_outs
    fn = jax.jit(shard_map(_body, mesh=mesh, in_specs=in_specs,
                           out_specs=out_specs, check_rep=False),
                 donate_argnums=tuple(range(n_params, n_params + n_outs)))

    def _zeros():
        return tuple(jnp.zeros((N_CORES * s[0], *s[1:]), d) for (s, d) in out_np)
    make_zeros = jax.jit(_zeros, out_shardings=(sh,) * n_outs)

    def run(in_maps):
        import jax as _jax
        concat_in = [
            _jax.device_put(
                np.concatenate([np.asarray(in_maps[c][nm]) for c in range(N_CORES)],
                               axis=0), sh)
            for nm in in_names
        ]
        outs = fn(*concat_in, *make_zeros())
        results = []
        host = [np.asarray(o) for o in outs]
        for c in range(N_CORES):
            results.append({nm: host[i].reshape(N_CORES, *out_np[i][0])[c]
                            for i, nm in enumerate(out_names)})
        return results

    return run


def kernel(x):
    import json
    import os

    x = np.asarray(x, dtype=np.float32)
    if "nc" not in _CACHE:
        kw = json.loads(os.environ.get("KERNEL_BUILD_KWARGS", "{}"))
        _CACHE["nc"] = build(reps=1, **kw)
    in_maps = _prepare_inputs(x)

    from concourse._compat import axon_active
    if axon_active():
        # axon PJRT proxy: use a cached jit(shard_map) runner so repeat calls
        # skip retrace/recompile (run_bass_kernel_spmd rebuilds its jit
        # closure per call).
        if "run" not in _CACHE:
            _CACHE["run"] = _build_runner(_CACHE["nc"])
        results = _CACHE["run"](in_maps)
    else:
        # native /dev/neuron* path
        from concourse.bass_utils import run_bass_kernel_spmd
        results = run_bass_kernel_spmd(_CACHE["nc"], in_maps,
                                       core_ids=list(range(N_CORES))).results
    return _gather_outputs(results)
